# revision 34
# baseline (speedup 1.0000x reference)
"""Deformable-DETR transformer encoder layer on 8 Trainium2 NeuronCores.

Sharding: data-parallel over batch (B=2) x 4-way sequence-parallel over query
tokens. Each core builds the full multiscale value maps for its batch
(redundant within the 4-core group so the deformable gather stays local),
then processes its 1/4 shard of queries through sampling + attention + FFN.

Device pipeline per core (channel-major activations [C, T]; x = feat+pos is
pre-added on the host and shipped bf16):
  1. v = x @ W_val (bf16 PE) builds the "quad" map vq[pos, m, 128] whose 256B
     rows each hold a 2x2 bilinear patch per head, written DIRECTLY from SBUF:
     level 0 assembles each quad row band in SBUF (PE shift-matmul supplies the
     x-1 corners) and writes one contiguous 256KB DMA per image row; smaller
     levels use 4 strided corner writes per value chunk.
  2. Per query supertile: offsets/attention logits via PE (W_off columns
     permuted so row s = m*16+l*4+k of the transposed output is sample s's
     px/py); softmax via exp + ones-matmul group sums; bilinear weights,
     in-bounds masks and int16 gather indices on DVE.
  3. dma_gather (GPSIMD SWDGE) fetches the 2x2 patches, calls rotating over
     the 4 SWDGE queues (queue = global call index % 4 so the tile framework's
     round-robin DMASW lane assignment stays queue-consistent; each queue runs
     on its own Q7 core pair); DVE multiplies by the 4 corner weights
     (premultiplied by attention) and tree-reduces.
  4. W_out projection + residual + LN (mean/var via ones-matmuls) + FFN + LN.
"""

import numpy as np
import ml_dtypes

C, M, KPT, L, D = 256, 8, 4, 4, 32
B = 2
SIZES = [(128, 128), (64, 64), (32, 32), (16, 16)]
EPS = 1e-5
NCORES = 8
QSHARDS = 4

F32 = np.float32
BF16 = ml_dtypes.bfloat16


def _geom(sizes):
    hw = [h * w for h, w in sizes]
    ntok = sum(hw)
    lvl_base = np.cumsum([0] + hw).tolist()
    p_rows = [(h + 2) * (w + 2) for h, w in sizes]
    pb = np.cumsum([0] + p_rows).tolist()
    q_rows = [(h + 1) * (w + 1) for h, w in sizes]
    qb = np.cumsum([0] + q_rows).tolist()
    return hw, ntok, lvl_base, p_rows, pb[:-1], pb[-1], q_rows, qb[:-1], qb[-1]


HWL, NTOK, LVL_BASE, P_ROWS, P_BASE, P_TOT, Q_ROWS, Q_BASE, Q_TOT = _geom(SIZES)
BAND_LV0 = True   # level-0 quad rows assembled in SBUF, one DMA per row
QC_CORE = NTOK // QSHARDS              # 5440
QP = ((QC_CORE + 127) // 128) * 128    # 5504


def _supertiles(qp):
    # small first supertile: its weight math gates the first dma_gather, so
    # a short st0 gets the Pool engine (the bottleneck) going ~4x sooner
    ch = qp // 128
    out = []
    first = True
    while ch > 0:
        take = min(4 if first and ch > 8 else 15, ch)
        first = False
        out.append(take * 128)
        ch -= take
    return out


def build_program(sizes=None, qp=None, parts=("value", "v_zero", "v_proj", "v_corner", "query", "weights", "w_soft", "w_math", "w_beta", "w_idx", "gather", "tail"), debug=False):
    """Build the Bass program (same program for every core; SPMD over data)."""
    import concourse.mybir as mybir
    import concourse.tile as tile
    from concourse import bacc
    from concourse.masks import make_identity

    if sizes is None:
        sizes = SIZES
    if qp is None:
        qp = QP
    supertiles = _supertiles(qp)
    hwl, ntok, lvl_base, p_rows, p_base, p_tot, q_rows, q_base, q_tot = _geom(sizes)

    f32 = mybir.dt.float32
    bf16 = mybir.dt.bfloat16
    i16 = mybir.dt.int16
    AL = mybir.AluOpType
    AF = mybir.ActivationFunctionType

    nc = bacc.Bacc("TRN2", target_bir_lowering=False, debug=False,
                   num_swdge_queues=4)

    # ---------------- I/O ----------------
    xTb = nc.dram_tensor("xTb", (C, ntok), bf16, kind="ExternalInput")
    zfTq = nc.dram_tensor("zfTq", (C, qp), f32, kind="ExternalInput")
    refxb_d = nc.dram_tensor("refxb", (128, qp), f32, kind="ExternalInput")
    refyb_d = nc.dram_tensor("refyb", (128, qp), f32, kind="ExternalInput")
    consts_d = nc.dram_tensor("consts", (128, 8), f32, kind="ExternalInput")
    shup_d = nc.dram_tensor("shup", (128, 128), bf16, kind="ExternalInput")
    # consts cols: 0:W 1:H 2:W+1 3:W-1 4:H-1 5:W-2 6:H-2 7:unused
    wval_d = nc.dram_tensor("wval", (128, 2, C), bf16, kind="ExternalInput")
    woff_d = nc.dram_tensor("woff", (128, 2, C), bf16, kind="ExternalInput")
    wattn_d = nc.dram_tensor("wattn", (128, 2, 128), bf16, kind="ExternalInput")
    wout_d = nc.dram_tensor("wout", (128, 2, C), bf16, kind="ExternalInput")
    w1_d = nc.dram_tensor("w1", (128, 2, 2048), bf16, kind="ExternalInput")
    w2_d = nc.dram_tensor("w2", (128, 16, C), bf16, kind="ExternalInput")
    bval_bc_d = nc.dram_tensor("bval_bc", (128, C), f32, kind="ExternalInput")
    boffx_d = nc.dram_tensor("boffx", (128, 1), f32, kind="ExternalInput")  # b_off-0.5
    boffy_d = nc.dram_tensor("boffy", (128, 1), f32, kind="ExternalInput")
    battn_d = nc.dram_tensor("battn", (128, 1), f32, kind="ExternalInput")
    sones_d = nc.dram_tensor("sones", (128, 8), f32, kind="ExternalInput")
    sblk_d = nc.dram_tensor("sblk", (8, 128), f32, kind="ExternalInput")
    bout_d = nc.dram_tensor("bout", (128, 2), f32, kind="ExternalInput")
    b1_d = nc.dram_tensor("b1", (128, 16), f32, kind="ExternalInput")
    b2_d = nc.dram_tensor("b2", (128, 2), f32, kind="ExternalInput")
    g1_d = nc.dram_tensor("g1", (128, 2), f32, kind="ExternalInput")
    be1_d = nc.dram_tensor("be1", (128, 2), f32, kind="ExternalInput")
    g2_d = nc.dram_tensor("g2", (128, 2), f32, kind="ExternalInput")
    be2_d = nc.dram_tensor("be2", (128, 2), f32, kind="ExternalInput")
    outT = nc.dram_tensor("outT", (C, qp), f32, kind="ExternalOutput")
    if debug:
        dbg_vq = nc.dram_tensor("dbg_vq", (q_tot, M, 128), bf16, kind="ExternalOutput")
        dbg_idx = nc.dram_tensor("dbg_idx", (128, qp), i16, kind="ExternalOutput")
        dbg_beta = nc.dram_tensor("dbg_beta", (128, qp // 128, 4, 128), bf16, kind="ExternalOutput")
        dbg_acc = nc.dram_tensor("dbg_acc", (128, qp // 128, M, D), f32, kind="ExternalOutput")
        dbg_g = nc.dram_tensor("dbg_g", (128, 4 * (qp // 128), 128), bf16, kind="ExternalOutput")

    # DRAM scratch
    vq = nc.dram_tensor("vq", (q_tot, M, 128), bf16)

    with tile.TileContext(nc) as tc:
        with (
            tc.tile_pool(name="const", bufs=1) as cpool,
            tc.tile_pool(name="wpool", bufs=1) as wpool,
            tc.tile_pool(name="stp", bufs=1) as stpool,
            tc.tile_pool(name="dram", bufs=2, space="DRAM") as dpool,
        ):
            # ------------ constants / weights into SBUF ------------
            def load1(pool, dram, shape, dt):
                t = pool.tile(list(shape), dt, tag=dram.name, name=dram.name + "_sb")
                nc.sync.dma_start(t[:], dram[:])
                return t

            consts = load1(cpool, consts_d, (128, 8), f32)
            shup = load1(cpool, shup_d, (128, 128), bf16)
            W_row, H_row = consts[:, 0:1], consts[:, 1:2]
            Wp1_row = consts[:, 2:3]
            Wm1_row, Hm1_row = consts[:, 3:4], consts[:, 4:5]
            Wm2_row, Hm2_row = consts[:, 5:6], consts[:, 6:7]
            wval = load1(wpool, wval_d, (128, 2, C), bf16)
            woff = load1(wpool, woff_d, (128, 2, C), bf16)
            wattn = load1(wpool, wattn_d, (128, 2, 128), bf16)
            wout = load1(wpool, wout_d, (128, 2, C), bf16)
            w1 = load1(wpool, w1_d, (128, 2, 2048), bf16)
            w2 = load1(wpool, w2_d, (128, 16, C), bf16)
            bval_bc = load1(cpool, bval_bc_d, (128, C), f32)
            boffx = load1(cpool, boffx_d, (128, 1), f32)
            boffy = load1(cpool, boffy_d, (128, 1), f32)
            battn = load1(cpool, battn_d, (128, 1), f32)
            sones = load1(cpool, sones_d, (128, 8), f32)
            sblk = load1(cpool, sblk_d, (8, 128), f32)
            bout_t = load1(cpool, bout_d, (128, 2), f32)
            b1_t = load1(cpool, b1_d, (128, 16), f32)
            b2_t = load1(cpool, b2_d, (128, 2), f32)
            g1_t = load1(cpool, g1_d, (128, 2), f32)
            be1_t = load1(cpool, be1_d, (128, 2), f32)
            g2_t = load1(cpool, g2_d, (128, 2), f32)
            be2_t = load1(cpool, be2_d, (128, 2), f32)

            ident_bf = cpool.tile([128, 128], bf16)
            make_identity(nc, ident_bf[:])
            ident_f32 = cpool.tile([128, 128], f32)
            make_identity(nc, ident_f32[:])
            ones_col = cpool.tile([128, 1], f32)   # lhsT for column sums
            nc.vector.memset(ones_col[:], 1.0)
            ones_row = cpool.tile([1, 128], f32)   # lhsT for k=1 bcast
            nc.vector.memset(ones_row[:], 1.0)
            zt = cpool.tile([128, 2048], bf16)
            nc.vector.memset(zt[:], 0.0)
            eps1 = cpool.tile([1, 1], f32)
            nc.vector.memset(eps1[:], EPS)

            # ============ Phase 1: values -> quad map (direct) ============
            # quad row (qy,qx) slot ci=(dy,dx) holds value[qy-1+dy, qx-1+dx];
            # each 128-token value chunk is written 4x straight into vq, and
            # only the quad border rows/cols (which reference OOB values) are
            # pre-zeroed.
            with (
                tc.tile_pool(name="vph", bufs=3) as vpool,
                tc.tile_pool(name="psV", bufs=3, space="PSUM") as psV,
            ):
                for lv, (H, W) in enumerate(sizes if "v_zero" in parts else []):
                    qm = vq[q_base[lv] : q_base[lv] + q_rows[lv]].rearrange(
                        "(y x) m e -> y x m e", x=W + 1
                    )
                    zrow = zt[:, :1024].rearrange("p (m e) -> p m e", e=128)
                    if BAND_LV0 and lv == 0 and W == 128:
                        # band-assembled level: only the x=W column rows need
                        # pre-zeroing (bands write full rows x=0..W-1)
                        for r0 in range(0, H + 1, 128):
                            rw = min(128, H + 1 - r0)
                            nc.sync.dma_start(qm[r0 : r0 + rw, W], zrow[:rw])
                        continue
                    for band in (qm[0], qm[H]):          # top / bottom rows
                        for r0 in range(0, W + 1, 128):
                            rw = min(128, W + 1 - r0)
                            nc.sync.dma_start(band[r0 : r0 + rw], zrow[:rw])
                    for band in (qm[1:H, 0], qm[1:H, W]):  # left / right cols
                        if H > 1:
                            nc.sync.dma_start(band, zrow[: H - 1])

                for lv, (H, W) in enumerate(sizes if "v_proj" in parts else []):
                    hwt = hwl[lv]
                    qm = vq[q_base[lv] : q_base[lv] + q_rows[lv]].rearrange(
                        "(y x) m e -> y x m e", x=W + 1
                    )
                    band_mode = BAND_LV0 and lv == 0 and W == 128
                    vbt_prev = None   # value row y-1 (band mode)
                    sh_prev = None    # x-shifted value row y-1

                    def emit_band(y, v_prev, s_prev, v_cur, s_cur):
                        """Write quad rows (y, x=0..W-1) as one contiguous DMA.
                        slot ci0=(0,0)<-s_prev ci1=(0,1)<-v_prev
                             ci2=(1,0)<-s_cur  ci3=(1,1)<-v_cur"""
                        Q = vpool.tile([128, M, 4, D], bf16, tag="qband",
                                       name="qband", bufs=2)
                        vv = lambda t: t[:].rearrange("p (m d) -> p m d", d=D)
                        nc.vector.tensor_copy(Q[:, :, 1, :], vv(v_prev))
                        nc.vector.tensor_copy(Q[:, :, 3, :], vv(v_cur))
                        nc.scalar.copy(Q[:, :, 0, :], vv(s_prev))
                        nc.scalar.copy(Q[:, :, 2, :], vv(s_cur))
                        r0 = q_base[0] + y * (W + 1)
                        nc.sync.dma_start(vq[r0 : r0 + W], Q[:])

                    TT = min(512, hwt)
                    for t0 in range(0, hwt, TT):
                        tt_ = min(TT, hwt - t0)
                        xb = vpool.tile([128, 2, TT], bf16, tag="xb", name="xb")
                        nc.sync.dma_start(
                            xb[:, :, :tt_],
                            xTb[:, lvl_base[lv] + t0 : lvl_base[lv] + t0 + tt_]
                            .rearrange("(co ci) t -> ci co t", ci=128),
                        )
                        for c0 in range(0, tt_, 128):
                            cw = min(128, tt_ - c0)
                            pv = psV.tile([128, C], f32, tag="psv", name="psv")
                            vps = pv[:cw, :]
                            for co in range(2):
                                nc.tensor.matmul(
                                    vps, xb[:, co, c0 : c0 + cw], wval[:, co, :],
                                    start=(co == 0), stop=(co == 1),
                                )
                            vbt = vpool.tile([128, C], bf16, tag="vbt", name="vbt",
                                             bufs=3)
                            nc.vector.tensor_tensor(vbt[:cw], vps, bval_bc[:cw], AL.add)
                            tglob = t0 + c0
                            vy0 = tglob // W
                            assert tglob % W == 0 and cw % W == 0
                            nr = cw // W
                            if band_mode:
                                # x-shift via PE: sh[p] = vbt[p-1]
                                psS = psV.tile([128, C], f32, tag="pss", name="pss")
                                nc.tensor.matmul(psS[:], shup[:], vbt[:])
                                sh_cur = vpool.tile([128, C], bf16, tag="sh",
                                                    name="sh", bufs=2)
                                nc.scalar.copy(sh_cur[:], psS[:])
                                zrow_c = zt[:, :C]
                                emit_band(
                                    vy0,
                                    vbt_prev if vbt_prev is not None else zrow_c,
                                    sh_prev if sh_prev is not None else zrow_c,
                                    vbt, sh_cur,
                                )
                                # x=W column rows get value (y, W-1) = vbt[127]:
                                # (qy=y, ci2) and (qy=y+1, ci0)
                                src_c = vbt[127:128].rearrange(
                                    "p (m d) -> p m d", d=D)
                                nc.sync.dma_start(
                                    qm[vy0, W, :, 2 * D : 3 * D], src_c)
                                nc.sync.dma_start(
                                    qm[vy0 + 1, W, :, 0:D], src_c)
                                vbt_prev, sh_prev = vbt, sh_cur
                                continue
                            for ci, (dy, dx) in enumerate(
                                ((0, 0), (0, 1), (1, 0), (1, 1))
                            ):
                                dst = qm[
                                    vy0 + 1 - dy : vy0 + 1 - dy + nr,
                                    1 - dx : 1 - dx + W,
                                    :,
                                    ci * D : (ci + 1) * D,
                                ]
                                nc.sync.dma_start(dst, vbt[:cw])
                    if band_mode:
                        # last band y=H uses V_H = 0
                        emit_band(H, vbt_prev, sh_prev, zt[:, :C], zt[:, :C])

            if debug:
                nc.sync.dma_start(dbg_vq[:], vq[:])

            # keepalive reads for partial (bisect) builds so dead-allocation
            # removal doesn't drop DRAM scratch still referenced by DMAs
            full = all(p in parts for p in ("value", "weights", "gather", "tail"))
            if not full:
                ka = cpool.tile([1, 4], f32, tag="ka", name="ka")
                kb = cpool.tile([1, 4], bf16, tag="kb", name="kb")
                nc.sync.dma_start(kb[0:1, 0:2], vq[1:2, 0, 0:2])
                nc.sync.dma_start(kb[0:1, 2:4], vq[0:1, 0, 0:2])
                nc.vector.tensor_copy(ka[:], kb[:])
                nc.sync.dma_start(outT[0:1, 0:4], ka[:])

            # ============ Phase 2: query supertiles ============
            # global Pool-DMA instruction counter: the tile framework assigns
            # DMASW sem lanes round-robin mod 8 over these instructions, and a
            # lane must stay on one SWDGE queue -> queue = counter % 4 keeps
            # lane l on queue l%4 consistently
            gq_counter = [0]
            st_off = 0
            for sti, qst in enumerate(supertiles if "query" in parts else []):
                QCh = qst // 128
                q_sl = slice(st_off, st_off + qst)

                # betaT/idx16/zf double-buffered so supertile i+1's weight
                # math can land while supertile i's gathers still read them
                zfT = stpool.tile([128, 2, qst], f32, tag="zfT", name="zfT")
                zfb = stpool.tile([128, 2, qst], bf16, tag="zfb", name="zfb",
                                  bufs=2)
                betaT = stpool.tile([128, QCh, 4, 128], bf16, tag="betaT",
                                    name="betaT", bufs=2)
                idx16 = stpool.tile([128, qst], i16, tag="idx16", name="idx16",
                                    bufs=2)
                acc = stpool.tile([128, QCh, M, D], f32, tag="acc", name="acc")
                accT = stpool.tile([128, 2, qst], bf16, tag="accT", name="accT")

                # ---- zf (feat+pos pre-added on host) ----
                nc.sync.dma_start(
                    zfT[:], zfTq[:, q_sl].rearrange("(co ci) t -> ci co t", ci=128)
                )
                nc.vector.tensor_copy(zfb[:], zfT[:])

                # ---- weight math: psum-coupled per-512 loop, then
                # full-supertile DVE ops with aggressive buffer reuse ----
                with (
                    tc.tile_pool(name="wm", bufs=1) as mp,
                    tc.tile_pool(name="psQ", bufs=2, space="PSUM") as psQ,
                    tc.tile_pool(name="psW", bufs=2, space="PSUM") as psW,
                ):
                  if "weights" in parts:
                    def ft(tag, dt=f32):
                        return mp.tile([128, qst], dt, tag=tag, name=tag)

                    bx, by, At = ft("bx"), ft("by"), ft("At")
                    r1, r2 = ft("r1"), ft("r2")
                    t1, t2, t3, t4 = ft("t1"), ft("t2"), ft("t3"), ft("t4")
                    V = nc.vector

                    for qq in range(0, qst, 512):
                        qw = min(512, qst - qq)
                        sl = slice(qq, qq + qw)
                        for dst_t, j0, bias_t in ((bx, 0, boffx), (by, 128, boffy)):
                            ps = psQ.tile([128, 512], f32, tag="psq", name="psq")
                            for co in range(2):
                                nc.tensor.matmul(
                                    ps[:, :qw], woff[:, co, j0 : j0 + 128],
                                    zfb[:, co, sl], start=(co == 0), stop=(co == 1),
                                )
                            nc.scalar.activation(
                                dst_t[:, sl], ps[:, :qw], AF.Identity, bias=bias_t[:]
                            )
                        ps = psQ.tile([128, 512], f32, tag="psq", name="psq")
                        for co in range(2):
                            nc.tensor.matmul(
                                ps[:, :qw], wattn[:, co, :], zfb[:, co, sl],
                                start=(co == 0), stop=(co == 1),
                            )
                        nc.scalar.activation(At[:, sl], ps[:, :qw], AF.Exp, bias=battn[:])
                        gs = psW.tile([8, 512], f32, tag="gs", name="gs")
                        nc.tensor.matmul(gs[:, :qw], sones[:], At[:, sl])
                        rgs = mp.tile([8, 512], f32, tag="rgs", name="rgs")
                        nc.vector.reciprocal(rgs[:, :qw], gs[:, :qw])
                        rb = psW.tile([128, 512], f32, tag="rb", name="rb")
                        nc.tensor.matmul(rb[:, :qw], sblk[:], rgs[:, :qw])
                        V.tensor_tensor(At[:, sl], At[:, sl], rb[:, :qw], AL.mult)

                    # refs (full supertile)
                    nc.sync.dma_start(r1[:], refxb_d[:, q_sl])
                    nc.sync.dma_start(r2[:], refyb_d[:, q_sl])
                    # px/py
                    V.scalar_tensor_tensor(bx[:], r1[:], W_row, bx[:], AL.mult, AL.add)
                    V.scalar_tensor_tensor(by[:], r2[:], H_row, by[:], AL.mult, AL.add)
                    BIG = float(3 << 22)
                    # x0f -> t1 (round(px-0.5) via magic adds), wx -> r1
                    V.tensor_scalar(t1[:], bx[:], -0.5, None, AL.add)
                    V.tensor_scalar(t1[:], t1[:], BIG, None, AL.add)
                    V.tensor_scalar(t1[:], t1[:], -BIG, None, AL.add)
                    V.tensor_tensor(r1[:], bx[:], t1[:], AL.subtract)
                    # y0f -> t2, wy -> r2
                    V.tensor_scalar(t2[:], by[:], -0.5, None, AL.add)
                    V.tensor_scalar(t2[:], t2[:], BIG, None, AL.add)
                    V.tensor_scalar(t2[:], t2[:], -BIG, None, AL.add)
                    V.tensor_tensor(r2[:], by[:], t2[:], AL.subtract)
                    # mx0 -> bx, mx1 -> by
                    V.tensor_scalar(bx[:], t1[:], 0.0, None, AL.is_ge)
                    V.tensor_scalar(t3[:], t1[:], Wm1_row, None, AL.is_le)
                    V.tensor_tensor(bx[:], bx[:], t3[:], AL.mult)
                    V.tensor_scalar(by[:], t1[:], -1.0, None, AL.is_ge)
                    V.tensor_scalar(t3[:], t1[:], Wm2_row, None, AL.is_le)
                    V.tensor_tensor(by[:], by[:], t3[:], AL.mult)
                    # u0 -> bx, u1 -> by
                    V.tensor_scalar(t3[:], r1[:], -1.0, 1.0, AL.mult, AL.add)
                    V.tensor_tensor(bx[:], t3[:], bx[:], AL.mult)
                    V.tensor_tensor(by[:], r1[:], by[:], AL.mult)
                    # my0 -> r1, my1 -> t4
                    V.tensor_scalar(r1[:], t2[:], 0.0, None, AL.is_ge)
                    V.tensor_scalar(t3[:], t2[:], Hm1_row, None, AL.is_le)
                    V.tensor_tensor(r1[:], r1[:], t3[:], AL.mult)
                    V.tensor_scalar(t4[:], t2[:], -1.0, None, AL.is_ge)
                    V.tensor_scalar(t3[:], t2[:], Hm2_row, None, AL.is_le)
                    V.tensor_tensor(t4[:], t4[:], t3[:], AL.mult)
                    # v0 -> r1, v1 -> t4
                    V.tensor_scalar(t3[:], r2[:], -1.0, 1.0, AL.mult, AL.add)
                    V.tensor_tensor(r1[:], t3[:], r1[:], AL.mult)
                    V.tensor_tensor(t4[:], r2[:], t4[:], AL.mult)
                    # betas (bf16) and transposes into betaT
                    bbs = []
                    for ci, (uu, vv) in enumerate(
                        ((bx, r1), (by, r1), (bx, t4), (by, t4))
                    ):
                        bb = mp.tile([128, qst], bf16, tag=f"bb{ci}", name=f"bb{ci}")
                        V.tensor_tensor(t3[:], uu[:], vv[:], AL.mult)
                        V.tensor_tensor(bb[:], t3[:], At[:], AL.mult)
                        bbs.append(bb)
                    if "w_beta" in parts:
                        for ci in range(4):
                            for qc in range(QCh):
                                pst = psW.tile([128, 128], bf16, tag="pst", name="pst")
                                nc.tensor.transpose(
                                    pst[:], bbs[ci][:, qc * 128 : (qc + 1) * 128],
                                    ident_bf[:],
                                )
                                nc.scalar.copy(betaT[:, qc, ci, :], pst[:])
                    # x0p -> t1, y0p -> t2, idx
                    V.tensor_scalar(t1[:], t1[:], 1.0, 0.0, AL.add, AL.max)
                    V.tensor_scalar(t1[:], t1[:], W_row, None, AL.min)
                    V.tensor_scalar(t2[:], t2[:], 1.0, 0.0, AL.add, AL.max)
                    V.tensor_scalar(t2[:], t2[:], H_row, None, AL.min)
                    V.scalar_tensor_tensor(t3[:], t2[:], Wp1_row, t1[:], AL.mult, AL.add)
                    V.tensor_copy(idx16[:], t3[:])

                # ---- gather + combine per (level, head) ----
                nc.vector.memset(acc[:], 0.0)
                JJ = 4 * qst
                FF = JJ // 16          # idx cols per (lv, m)
                RL = 8 * FF            # dlin rows per level (m-major, then k, f)
                if "gather" in parts:
                    idxd = dpool.tile([128, qst], i16, tag="idxd", name="idxd")
                    nc.sync.dma_start(idxd[:], idx16[:])
                    # 16-wrapped index image, all (lv, m) pairs. The SWDGE
                    # ucode pair for queue q only reads idx partitions
                    # 32q..32q+31 (replicas r=2q,2q+1); CoreSim always checks
                    # r=0,1. Write only those replicas instead of all 8.
                    # (DMA AP balancer caps at 3 dims -> one DMA per slice)
                    dlin_all = dpool.tile([L, RL, 128], i16, tag="dlin", name="dlin")
                    for lv in range(L):
                        for m in range(M):
                            s0 = m * 16 + lv * 4
                            srcb = idxd[s0 : s0 + 4].rearrange(
                                "k (f ql) -> (k f) ql", ql=16
                            )
                            dst3 = dlin_all[lv, m * FF : (m + 1) * FF].rearrange(
                                "f (r ql) -> f r ql", r=8
                            )
                            # queues 0-3 rotate per call: all 8 replicas
                            nc.sync.dma_start(
                                dst3, srcb[:, None, :].to_broadcast((FF, 8, 16))
                            )
                with tc.tile_pool(name="gp", bufs=2) as gp:
                    for lv in range(L if "gather" in parts else 0):
                        idxw = gp.tile([128, RL], i16, tag="idxw", name="idxw",
                                       bufs=1)
                        nc.sync.dma_start_transpose(idxw[:], dlin_all[lv])
                        for m in range(M):
                            s0 = m * 16 + lv * 4
                            ib = m * FF
                            g = gp.tile([128, 4 * QCh, 128], bf16, tag="g", name="g")
                            # SWDGE descriptor ring holds 1024 descs: split
                            # into <=1024-index sub-calls (128-aligned),
                            # rotating across the 4 SWDGE queues.
                            for c0 in range(0, JJ, 1024):
                                n_i = min(1024, JJ - c0)
                                nc.gpsimd.dma_gather(
                                    out_ap=g[:, c0 // 128 : (c0 + n_i) // 128, :],
                                    in_ap=vq[q_base[lv] : q_base[lv] + q_rows[lv], m, :],
                                    idxs_ap=idxw[:, ib + c0 // 16 : ib + (c0 + n_i) // 16],
                                    num_idxs=n_i,
                                    num_idxs_reg=n_i,
                                    elem_size=128,
                                    elem_step=M * 128,
                                    queue_num=gq_counter[0] % 4,
                                )
                                gq_counter[0] += 1
                            if debug and sti == 0 and lv == 0 and m == 0:
                                nc.sync.dma_start(dbg_g[:, : 4 * QCh, :], g[:])
                            gv = g[:].rearrange(
                                "p (k qc) (c d) -> p k qc c d", k=4, d=D
                            )
                            bt = betaT[:, :, :, s0 : s0 + 4]
                            btv = bt.rearrange("p qc c k -> p k qc c")[
                                :, :, :, :, None
                            ].to_broadcast((128, 4, QCh, 4, D))
                            tmp = gp.tile([128, 4, QCh, 4, D], bf16, tag="tmp", name="tmp", bufs=1)
                            nc.vector.tensor_tensor(tmp[:], gv, btv, AL.mult)
                            s1 = gp.tile([128, 4, QCh, 2, D], bf16, tag="s1", name="s1", bufs=1)
                            nc.vector.tensor_tensor(
                                s1[:], tmp[:, :, :, 0:4:2, :], tmp[:, :, :, 1:4:2, :],
                                AL.add,
                            )
                            s2 = gp.tile([128, 4, QCh, D], bf16, tag="s2", name="s2", bufs=1)
                            nc.vector.tensor_tensor(
                                s2[:], s1[:, :, :, 0, :], s1[:, :, :, 1, :], AL.add
                            )
                            s3 = gp.tile([128, 2, QCh, D], bf16, tag="s3", name="s3", bufs=1)
                            nc.vector.tensor_tensor(
                                s3[:], s2[:, 0:4:2], s2[:, 1:4:2], AL.add
                            )
                            s4 = gp.tile([128, QCh, D], f32, tag="s4", name="s4", bufs=1)
                            nc.vector.tensor_tensor(s4[:], s3[:, 0], s3[:, 1], AL.add)
                            nc.vector.tensor_tensor(
                                acc[:, :, m, :], acc[:, :, m, :], s4[:], AL.add
                            )

                if debug and sti == 0:
                    nc.sync.dma_start(dbg_idx[:, :qst], idx16[:])
                    nc.sync.dma_start(dbg_beta[:, :QCh], betaT[:])
                    nc.sync.dma_start(dbg_acc[:, :QCh], acc[:])

                # ---- transpose acc to channel-major bf16 ----
                with tc.tile_pool(name="psX", bufs=2, space="PSUM") as psX:
                    accv = acc[:].rearrange("p qc m d -> p qc (m d)")
                    for qc in range(QCh if "tail" in parts else 0):
                        for jb in range(2):
                            pst2 = psX.tile([128, 128], f32, tag="pst2", name="pst2")
                            nc.tensor.transpose(
                                pst2[:], accv[:, qc, jb * 128 : (jb + 1) * 128],
                                ident_f32[:],
                            )
                            nc.scalar.copy(
                                accT[:, jb, qc * 128 : (qc + 1) * 128], pst2[:]
                            )

                # ---- out proj + residual + LN1 + FFN + LN2 ----
                with (
                    tc.tile_pool(name="fp", bufs=2) as fp,
                    tc.tile_pool(name="lnp", bufs=1) as lp,
                    tc.tile_pool(name="psF", bufs=3, space="PSUM") as psF,
                    tc.tile_pool(name="psL", bufs=1, space="PSUM") as psL,
                ):
                    def layernorm(x_t, g_col, be_col, dst_f32, dst_bf, qw):
                        """x_t: [128, 2, qw] fp32 -> dst tiles [128, 2, qw]."""
                        mu = psL.tile([1, 512], f32, tag="mu", name="mu")
                        for co in range(2):
                            nc.tensor.matmul(
                                mu[:, :qw], ones_col[:], x_t[:, co, :qw],
                                start=(co == 0), stop=(co == 1),
                            )
                        mus = lp.tile([1, 512], f32, tag="mus", name="mus")
                        nc.scalar.activation(
                            mus[:, :qw], mu[:, :qw], AF.Identity, scale=1.0 / C
                        )
                        mub = psL.tile([128, 512], f32, tag="mub", name="mub")
                        nc.tensor.matmul(mub[:, :qw], ones_row[:], mus[:, :qw])
                        xc = lp.tile([128, 2, 512], f32, tag="xc", name="xc")
                        sq = lp.tile([128, 2, 512], f32, tag="sq", name="sq")
                        for co in range(2):
                            nc.vector.tensor_tensor(
                                xc[:, co, :qw], x_t[:, co, :qw], mub[:, :qw],
                                AL.subtract,
                            )
                            nc.scalar.activation(
                                sq[:, co, :qw], xc[:, co, :qw], AF.Square
                            )
                        var = psL.tile([1, 512], f32, tag="var", name="var")
                        for co in range(2):
                            nc.tensor.matmul(
                                var[:, :qw], ones_col[:], sq[:, co, :qw],
                                start=(co == 0), stop=(co == 1),
                            )
                        sd = lp.tile([1, 512], f32, tag="sd", name="sd")
                        nc.scalar.activation(
                            sd[:, :qw], var[:, :qw], AF.Sqrt, bias=eps1[:], scale=1.0 / C
                        )
                        rsd = lp.tile([1, 512], f32, tag="rsd", name="rsd")
                        nc.vector.reciprocal(rsd[:, :qw], sd[:, :qw])
                        isb = psL.tile([128, 512], f32, tag="isb", name="isb")
                        nc.tensor.matmul(isb[:, :qw], ones_row[:], rsd[:, :qw])
                        for co in range(2):
                            nc.vector.tensor_tensor(
                                xc[:, co, :qw], xc[:, co, :qw], isb[:, :qw], AL.mult
                            )
                            nc.vector.tensor_scalar(
                                dst_f32[:, co, :qw], xc[:, co, :qw],
                                g_col[:, co : co + 1], be_col[:, co : co + 1],
                                AL.mult, AL.add,
                            )
                            if dst_bf is not None:
                                nc.vector.tensor_copy(
                                    dst_bf[:, co, :qw], dst_f32[:, co, :qw]
                                )

                    for qq in range(0, qst if "tail" in parts else 0, 512):
                        qw = min(512, qst - qq)
                        sl = slice(qq, qq + qw)
                        # x = zf + acc @ W_out + b_out
                        xT_t = fp.tile([128, 2, 512], f32, tag="xT_t", name="xT_t")
                        for jb in range(2):
                            ps = psF.tile([128, 512], f32, tag="psf", name="psf")
                            for co in range(2):
                                nc.tensor.matmul(
                                    ps[:, :qw],
                                    wout[:, co, jb * 128 : (jb + 1) * 128],
                                    accT[:, co, sl],
                                    start=(co == 0), stop=(co == 1),
                                )
                            nc.vector.scalar_tensor_tensor(
                                xT_t[:, jb, :qw], ps[:, :qw],
                                bout_t[:, jb : jb + 1], zfT[:, jb, sl],
                                AL.add, AL.add,
                            )
                        x1 = fp.tile([128, 2, 512], f32, tag="x1", name="x1")
                        x1b = fp.tile([128, 2, 512], bf16, tag="x1b", name="x1b")
                        layernorm(xT_t, g1_t, be1_t, x1, x1b, qw)

                        hb = fp.tile([128, 16, 512], bf16, tag="hb", name="hb")
                        for jb in range(16):
                            ps = psF.tile([128, 512], f32, tag="psf", name="psf")
                            for co in range(2):
                                nc.tensor.matmul(
                                    ps[:, :qw],
                                    w1[:, co, jb * 128 : (jb + 1) * 128],
                                    x1b[:, co, :qw],
                                    start=(co == 0), stop=(co == 1),
                                )
                            nc.scalar.activation(
                                hb[:, jb, :qw], ps[:, :qw], AF.Relu,
                                bias=b1_t[:, jb : jb + 1],
                            )
                        x2 = fp.tile([128, 2, 512], f32, tag="x2", name="x2")
                        for jb in range(2):
                            ps = psF.tile([128, 512], f32, tag="psf", name="psf")
                            for kb in range(16):
                                nc.tensor.matmul(
                                    ps[:, :qw],
                                    w2[:, kb, jb * 128 : (jb + 1) * 128],
                                    hb[:, kb, :qw],
                                    start=(kb == 0), stop=(kb == 15),
                                )
                            nc.vector.scalar_tensor_tensor(
                                x2[:, jb, :qw], ps[:, :qw], b2_t[:, jb : jb + 1],
                                x1[:, jb, :qw], AL.add, AL.add,
                            )
                        out5 = fp.tile([128, 2, 512], f32, tag="out5", name="out5")
                        layernorm(x2, g2_t, be2_t, out5, None, qw)
                        nc.sync.dma_start(
                            outT[:, st_off + qq : st_off + qq + qw].rearrange(
                                "(co ci) t -> ci co t", ci=128
                            ),
                            out5[:, :, :qw],
                        )

                st_off += qst

    nc.finalize()
    return nc


# ======================= host side =======================

def _prep_core_inputs(inputs, b, s, sizes=None, qp=None):
    """Build the per-core input map (numpy) for batch b, query shard s."""
    if sizes is None:
        sizes = SIZES
    if qp is None:
        qp = QP
    hwl, ntok, lvl_base, *_ = _geom(sizes)
    nl = len(sizes)

    feats = [np.asarray(inputs[f"feat{i}"]) for i in range(nl)]
    poss = [np.asarray(inputs[f"pos{i}"]) for i in range(nl)]
    refs = [np.asarray(inputs[f"ref{i}"]) for i in range(nl)]

    x_all = np.concatenate([f[b].reshape(-1, C) for f in feats], 0)   # [ntok, C]
    p_all = np.concatenate([p[b].reshape(-1, C) for p in poss], 0)
    xT = np.ascontiguousarray((x_all + p_all).T.astype(F32))
    xTb = xT.astype(BF16)

    own = []
    for i in range(nl):
        n4 = hwl[i] // QSHARDS
        own.append(np.arange(lvl_base[i] + s * n4, lvl_base[i] + (s + 1) * n4))
    own = np.concatenate(own)
    nq = own.shape[0]

    zfTq = np.zeros((C, qp), F32)
    zfTq[:, :nq] = xT[:, own]

    ref_all = np.concatenate([r[b].reshape(-1, 2) for r in refs], 0)
    refq = np.zeros((qp, 2), F32)
    refq[:nq] = ref_all[own]
    refxb = np.ascontiguousarray(np.broadcast_to(refq[:, 0], (128, qp))).astype(F32)
    refyb = np.ascontiguousarray(np.broadcast_to(refq[:, 1], (128, qp))).astype(F32)

    consts = np.zeros((128, 8), F32)
    for sr in range(128):
        lvl = (sr // KPT) % len(sizes)
        H, W = sizes[lvl]
        consts[sr] = [W, H, W + 1, W - 1, H - 1, W - 2, H - 2, 0]

    def t_in(w):  # [C, N] -> [128, 2, N] (ci, co, n) in bf16
        w = np.asarray(w)
        return np.ascontiguousarray(
            w.reshape(2, 128, -1).transpose(1, 0, 2)
        ).astype(BF16)

    W_off = np.asarray(inputs["W_off"]).reshape(C, M, L, KPT, 2)
    W_off_p = W_off.transpose(0, 4, 1, 2, 3).reshape(C, C)   # j' = c*128 + (m,l,k)
    b_off = np.asarray(inputs["b_off"]).reshape(M, L, KPT, 2)
    b_off_p = b_off.transpose(3, 0, 1, 2).reshape(C)

    w2 = np.asarray(inputs["W2"])
    w2_t = np.ascontiguousarray(w2.reshape(16, 128, C).transpose(1, 0, 2)).astype(BF16)

    col2 = lambda v: np.ascontiguousarray(np.asarray(v).reshape(2, 128).T).astype(F32)
    sones = np.zeros((128, 8), F32)
    for sr in range(128):
        sones[sr, sr // 16] = 1.0
    sblk = np.ascontiguousarray(sones.T).astype(F32)

    return {
        "xTb": xTb, "zfTq": zfTq,
        "shup": np.eye(128, k=1).astype(BF16),
        "refxb": refxb, "refyb": refyb, "consts": consts,
        "wval": t_in(inputs["W_val"]), "woff": t_in(W_off_p),
        "wattn": t_in(inputs["W_attn"]), "wout": t_in(inputs["W_out"]),
        "w1": t_in(inputs["W1"]), "w2": w2_t,
        "bval_bc": np.ascontiguousarray(
            np.broadcast_to(np.asarray(inputs["b_val"]), (128, C))).astype(F32),
        "boffx": np.ascontiguousarray((b_off_p[:128] - 0.5).reshape(128, 1)).astype(F32),
        "boffy": np.ascontiguousarray((b_off_p[128:] - 0.5).reshape(128, 1)).astype(F32),
        "battn": np.ascontiguousarray(
            np.asarray(inputs["b_attn"]).reshape(128, 1)).astype(F32),
        "sones": sones, "sblk": sblk,
        "bout": col2(inputs["b_out"]),
        "b1": np.ascontiguousarray(
            np.asarray(inputs["b1"]).reshape(16, 128).T).astype(F32),
        "b2": col2(inputs["b2"]),
        "g1": col2(inputs["g1"]), "be1": col2(inputs["be1"]),
        "g2": col2(inputs["g2"]), "be2": col2(inputs["be2"]),
    }, own, nq


_NC_CACHE = {}


def get_program():
    if "main" not in _NC_CACHE:
        _NC_CACHE["main"] = build_program()
    return _NC_CACHE["main"]


def kernel(**inputs):
    from concourse.bass_utils import run_bass_kernel_spmd

    nc = get_program()
    in_maps = []
    metas = []
    for c in range(NCORES):
        b, s = c // QSHARDS, c % QSHARDS
        im, own, nq = _prep_core_inputs(inputs, b, s)
        in_maps.append(im)
        metas.append((b, own, nq))

    res = run_bass_kernel_spmd(nc, in_maps, core_ids=list(range(NCORES)))

    out = np.zeros((B, NTOK, C), F32)
    for c in range(NCORES):
        b, own, nq = metas[c]
        outT = res.results[c]["outT"]          # [C, QP]
        out[b, own, :] = outT[:, :nq].T
    return out



# revision 35
# speedup vs baseline: 1.0814x; 1.0814x over previous
"""Deformable-DETR transformer encoder layer on 8 Trainium2 NeuronCores.

Sharding: data-parallel over batch (B=2) x 4-way sequence-parallel over query
tokens. Each core builds the full multiscale value maps for its batch
(redundant within the 4-core group so the deformable gather stays local),
then processes its 1/4 shard of queries through sampling + attention + FFN.

Device pipeline per core (channel-major activations [C, T]; x = feat+pos is
pre-added on the host and shipped bf16):
  1. v = x @ W_val (bf16 PE) builds the "quad" map vq[pos, m, 128] whose 256B
     rows each hold a 2x2 bilinear patch per head, written DIRECTLY from SBUF:
     level 0 assembles each quad row band in SBUF (PE shift-matmul supplies the
     x-1 corners) and writes one contiguous 256KB DMA per image row; smaller
     levels use 4 strided corner writes per value chunk.
  2. Per query supertile: offsets/attention logits via PE (W_off columns
     permuted so row s = m*16+l*4+k of the transposed output is sample s's
     px/py); softmax via exp + ones-matmul group sums; bilinear weights,
     in-bounds masks and int16 gather indices on DVE.
  3. dma_gather (GPSIMD SWDGE) fetches the 2x2 patches, calls rotating over
     the 4 SWDGE queues (queue = global call index % 4 so the tile framework's
     round-robin DMASW lane assignment stays queue-consistent; each queue runs
     on its own Q7 core pair); DVE multiplies by the 4 corner weights
     (premultiplied by attention) and tree-reduces.
  4. W_out projection + residual + LN (mean/var via ones-matmuls) + FFN + LN.
"""

import numpy as np
import ml_dtypes

C, M, KPT, L, D = 256, 8, 4, 4, 32
B = 2
SIZES = [(128, 128), (64, 64), (32, 32), (16, 16)]
EPS = 1e-5
NCORES = 8
QSHARDS = 4

F32 = np.float32
BF16 = ml_dtypes.bfloat16


def _geom(sizes):
    hw = [h * w for h, w in sizes]
    ntok = sum(hw)
    lvl_base = np.cumsum([0] + hw).tolist()
    p_rows = [(h + 2) * (w + 2) for h, w in sizes]
    pb = np.cumsum([0] + p_rows).tolist()
    q_rows = [(h + 1) * (w + 1) for h, w in sizes]
    qb = np.cumsum([0] + q_rows).tolist()
    return hw, ntok, lvl_base, p_rows, pb[:-1], pb[-1], q_rows, qb[:-1], qb[-1]


HWL, NTOK, LVL_BASE, P_ROWS, P_BASE, P_TOT, Q_ROWS, Q_BASE, Q_TOT = _geom(SIZES)
BAND_LV0 = True   # level-0 quad rows assembled in SBUF, one DMA per row
QC_CORE = NTOK // QSHARDS              # 5440
QP = ((QC_CORE + 127) // 128) * 128    # 5504


def _supertiles(qp):
    # small first supertile: its weight math gates the first dma_gather, so
    # a short st0 gets the Pool engine (the bottleneck) going ~4x sooner
    ch = qp // 128
    out = []
    first = True
    while ch > 0:
        take = min(4 if first and ch > 8 else 15, ch)
        first = False
        out.append(take * 128)
        ch -= take
    return out


def build_program(sizes=None, qp=None, parts=("value", "v_zero", "v_proj", "v_corner", "query", "weights", "w_soft", "w_math", "w_beta", "w_idx", "gather", "tail"), debug=False):
    """Build the Bass program (same program for every core; SPMD over data)."""
    import concourse.mybir as mybir
    import concourse.tile as tile
    from concourse import bacc
    from concourse.masks import make_identity

    if sizes is None:
        sizes = SIZES
    if qp is None:
        qp = QP
    supertiles = _supertiles(qp)
    hwl, ntok, lvl_base, p_rows, p_base, p_tot, q_rows, q_base, q_tot = _geom(sizes)

    f32 = mybir.dt.float32
    bf16 = mybir.dt.bfloat16
    i16 = mybir.dt.int16
    AL = mybir.AluOpType
    AF = mybir.ActivationFunctionType

    nc = bacc.Bacc("TRN2", target_bir_lowering=False, debug=False,
                   num_swdge_queues=4)

    # ---------------- I/O ----------------
    xTb = nc.dram_tensor("xTb", (C, ntok), bf16, kind="ExternalInput")
    zfTq = nc.dram_tensor("zfTq", (C, qp), f32, kind="ExternalInput")
    refxb_d = nc.dram_tensor("refxb", (128, qp), f32, kind="ExternalInput")
    refyb_d = nc.dram_tensor("refyb", (128, qp), f32, kind="ExternalInput")
    consts_d = nc.dram_tensor("consts", (128, 8), f32, kind="ExternalInput")
    shup_d = nc.dram_tensor("shup", (128, 128), bf16, kind="ExternalInput")
    # consts cols: 0:W 1:H 2:W+1 3:W-1 4:H-1 5:W-2 6:H-2 7:unused
    wval_d = nc.dram_tensor("wval", (128, 2, C), bf16, kind="ExternalInput")
    woff_d = nc.dram_tensor("woff", (128, 2, C), bf16, kind="ExternalInput")
    wattn_d = nc.dram_tensor("wattn", (128, 2, 128), bf16, kind="ExternalInput")
    wout_d = nc.dram_tensor("wout", (128, 2, C), bf16, kind="ExternalInput")
    w1_d = nc.dram_tensor("w1", (128, 2, 2048), bf16, kind="ExternalInput")
    w2_d = nc.dram_tensor("w2", (128, 16, C), bf16, kind="ExternalInput")
    bval_bc_d = nc.dram_tensor("bval_bc", (128, C), f32, kind="ExternalInput")
    boffx_d = nc.dram_tensor("boffx", (128, 1), f32, kind="ExternalInput")  # b_off-0.5
    boffy_d = nc.dram_tensor("boffy", (128, 1), f32, kind="ExternalInput")
    battn_d = nc.dram_tensor("battn", (128, 1), f32, kind="ExternalInput")
    sones_d = nc.dram_tensor("sones", (128, 8), f32, kind="ExternalInput")
    sblk_d = nc.dram_tensor("sblk", (8, 128), f32, kind="ExternalInput")
    bout_d = nc.dram_tensor("bout", (128, 2), f32, kind="ExternalInput")
    b1_d = nc.dram_tensor("b1", (128, 16), f32, kind="ExternalInput")
    b2_d = nc.dram_tensor("b2", (128, 2), f32, kind="ExternalInput")
    g1_d = nc.dram_tensor("g1", (128, 2), f32, kind="ExternalInput")
    be1_d = nc.dram_tensor("be1", (128, 2), f32, kind="ExternalInput")
    g2_d = nc.dram_tensor("g2", (128, 2), f32, kind="ExternalInput")
    be2_d = nc.dram_tensor("be2", (128, 2), f32, kind="ExternalInput")
    outT = nc.dram_tensor("outT", (C, qp), f32, kind="ExternalOutput")
    if debug:
        dbg_vq = nc.dram_tensor("dbg_vq", (q_tot, M, 128), bf16, kind="ExternalOutput")
        dbg_idx = nc.dram_tensor("dbg_idx", (128, qp), i16, kind="ExternalOutput")
        dbg_beta = nc.dram_tensor("dbg_beta", (128, qp // 128, 4, 128), bf16, kind="ExternalOutput")
        dbg_acc = nc.dram_tensor("dbg_acc", (128, qp // 128, M, D), f32, kind="ExternalOutput")
        dbg_g = nc.dram_tensor("dbg_g", (128, 4 * (qp // 128), 128), bf16, kind="ExternalOutput")

    # DRAM scratch
    vq = nc.dram_tensor("vq", (q_tot, M, 128), bf16)

    with tile.TileContext(nc) as tc:
        with (
            tc.tile_pool(name="const", bufs=1) as cpool,
            tc.tile_pool(name="wpool", bufs=1) as wpool,
            tc.tile_pool(name="stp", bufs=1) as stpool,
            tc.tile_pool(name="dram", bufs=2, space="DRAM") as dpool,
        ):
            # ------------ constants / weights into SBUF ------------
            def load1(pool, dram, shape, dt):
                t = pool.tile(list(shape), dt, tag=dram.name, name=dram.name + "_sb")
                nc.sync.dma_start(t[:], dram[:])
                return t

            consts = load1(cpool, consts_d, (128, 8), f32)
            shup = load1(cpool, shup_d, (128, 128), bf16)
            W_row, H_row = consts[:, 0:1], consts[:, 1:2]
            Wp1_row = consts[:, 2:3]
            Wm1_row, Hm1_row = consts[:, 3:4], consts[:, 4:5]
            Wm2_row, Hm2_row = consts[:, 5:6], consts[:, 6:7]
            wval = load1(wpool, wval_d, (128, 2, C), bf16)
            woff = load1(wpool, woff_d, (128, 2, C), bf16)
            wattn = load1(wpool, wattn_d, (128, 2, 128), bf16)
            wout = load1(wpool, wout_d, (128, 2, C), bf16)
            w1 = load1(wpool, w1_d, (128, 2, 2048), bf16)
            w2 = load1(wpool, w2_d, (128, 16, C), bf16)
            bval_bc = load1(cpool, bval_bc_d, (128, C), f32)
            boffx = load1(cpool, boffx_d, (128, 1), f32)
            boffy = load1(cpool, boffy_d, (128, 1), f32)
            battn = load1(cpool, battn_d, (128, 1), f32)
            sones = load1(cpool, sones_d, (128, 8), f32)
            sblk = load1(cpool, sblk_d, (8, 128), f32)
            bout_t = load1(cpool, bout_d, (128, 2), f32)
            b1_t = load1(cpool, b1_d, (128, 16), f32)
            b2_t = load1(cpool, b2_d, (128, 2), f32)
            g1_t = load1(cpool, g1_d, (128, 2), f32)
            be1_t = load1(cpool, be1_d, (128, 2), f32)
            g2_t = load1(cpool, g2_d, (128, 2), f32)
            be2_t = load1(cpool, be2_d, (128, 2), f32)

            ident_bf = cpool.tile([128, 128], bf16)
            make_identity(nc, ident_bf[:])
            ident_f32 = cpool.tile([128, 128], f32)
            make_identity(nc, ident_f32[:])
            ones_col = cpool.tile([128, 1], f32)   # lhsT for column sums
            nc.vector.memset(ones_col[:], 1.0)
            ones_row = cpool.tile([1, 128], f32)   # lhsT for k=1 bcast
            nc.vector.memset(ones_row[:], 1.0)
            zt = cpool.tile([128, 2048], bf16)
            nc.vector.memset(zt[:], 0.0)
            eps1 = cpool.tile([1, 1], f32)
            nc.vector.memset(eps1[:], EPS)

            # ============ Phase 1: values -> quad map (direct) ============
            # quad row (qy,qx) slot ci=(dy,dx) holds value[qy-1+dy, qx-1+dx];
            # each 128-token value chunk is written 4x straight into vq, and
            # only the quad border rows/cols (which reference OOB values) are
            # pre-zeroed.
            with (
                tc.tile_pool(name="vph", bufs=3) as vpool,
                tc.tile_pool(name="psV", bufs=3, space="PSUM") as psV,
            ):
                for lv, (H, W) in enumerate(sizes if "v_zero" in parts else []):
                    qm = vq[q_base[lv] : q_base[lv] + q_rows[lv]].rearrange(
                        "(y x) m e -> y x m e", x=W + 1
                    )
                    zrow = zt[:, :1024].rearrange("p (m e) -> p m e", e=128)
                    if BAND_LV0 and lv == 0 and W == 128:
                        # band-assembled level: only the x=W column rows need
                        # pre-zeroing (bands write full rows x=0..W-1)
                        for r0 in range(0, H + 1, 128):
                            rw = min(128, H + 1 - r0)
                            nc.sync.dma_start(qm[r0 : r0 + rw, W], zrow[:rw])
                        continue
                    for band in (qm[0], qm[H]):          # top / bottom rows
                        for r0 in range(0, W + 1, 128):
                            rw = min(128, W + 1 - r0)
                            nc.sync.dma_start(band[r0 : r0 + rw], zrow[:rw])
                    for band in (qm[1:H, 0], qm[1:H, W]):  # left / right cols
                        if H > 1:
                            nc.sync.dma_start(band, zrow[: H - 1])

                for lv, (H, W) in enumerate(sizes if "v_proj" in parts else []):
                    hwt = hwl[lv]
                    qm = vq[q_base[lv] : q_base[lv] + q_rows[lv]].rearrange(
                        "(y x) m e -> y x m e", x=W + 1
                    )
                    band_mode = BAND_LV0 and lv == 0 and W == 128
                    vbt_prev = None   # value row y-1 (band mode)
                    sh_prev = None    # x-shifted value row y-1

                    def emit_band(y, v_prev, s_prev, v_cur, s_cur):
                        """Write quad rows (y, x=0..W-1) as one contiguous DMA.
                        slot ci0=(0,0)<-s_prev ci1=(0,1)<-v_prev
                             ci2=(1,0)<-s_cur  ci3=(1,1)<-v_cur"""
                        Q = vpool.tile([128, M, 4, D], bf16, tag="qband",
                                       name="qband", bufs=2)
                        vv = lambda t: t[:].rearrange("p (m d) -> p m d", d=D)
                        nc.vector.tensor_copy(Q[:, :, 1, :], vv(v_prev))
                        nc.vector.tensor_copy(Q[:, :, 3, :], vv(v_cur))
                        nc.scalar.copy(Q[:, :, 0, :], vv(s_prev))
                        nc.scalar.copy(Q[:, :, 2, :], vv(s_cur))
                        r0 = q_base[0] + y * (W + 1)
                        nc.sync.dma_start(vq[r0 : r0 + W], Q[:])

                    TT = min(512, hwt)
                    for t0 in range(0, hwt, TT):
                        tt_ = min(TT, hwt - t0)
                        xb = vpool.tile([128, 2, TT], bf16, tag="xb", name="xb")
                        nc.sync.dma_start(
                            xb[:, :, :tt_],
                            xTb[:, lvl_base[lv] + t0 : lvl_base[lv] + t0 + tt_]
                            .rearrange("(co ci) t -> ci co t", ci=128),
                        )
                        for c0 in range(0, tt_, 128):
                            cw = min(128, tt_ - c0)
                            pv = psV.tile([128, C], f32, tag="psv", name="psv")
                            vps = pv[:cw, :]
                            for co in range(2):
                                nc.tensor.matmul(
                                    vps, xb[:, co, c0 : c0 + cw], wval[:, co, :],
                                    start=(co == 0), stop=(co == 1),
                                )
                            vbt = vpool.tile([128, C], bf16, tag="vbt", name="vbt",
                                             bufs=3)
                            nc.vector.tensor_tensor(vbt[:cw], vps, bval_bc[:cw], AL.add)
                            tglob = t0 + c0
                            vy0 = tglob // W
                            assert tglob % W == 0 and cw % W == 0
                            nr = cw // W
                            if band_mode:
                                # x-shift via PE: sh[p] = vbt[p-1]
                                psS = psV.tile([128, C], f32, tag="pss", name="pss")
                                nc.tensor.matmul(psS[:], shup[:], vbt[:])
                                sh_cur = vpool.tile([128, C], bf16, tag="sh",
                                                    name="sh", bufs=2)
                                nc.scalar.copy(sh_cur[:], psS[:])
                                zrow_c = zt[:, :C]
                                emit_band(
                                    vy0,
                                    vbt_prev if vbt_prev is not None else zrow_c,
                                    sh_prev if sh_prev is not None else zrow_c,
                                    vbt, sh_cur,
                                )
                                # x=W column rows get value (y, W-1) = vbt[127]:
                                # (qy=y, ci2) and (qy=y+1, ci0)
                                src_c = vbt[127:128].rearrange(
                                    "p (m d) -> p m d", d=D)
                                nc.sync.dma_start(
                                    qm[vy0, W, :, 2 * D : 3 * D], src_c)
                                nc.sync.dma_start(
                                    qm[vy0 + 1, W, :, 0:D], src_c)
                                vbt_prev, sh_prev = vbt, sh_cur
                                continue
                            for ci, (dy, dx) in enumerate(
                                ((0, 0), (0, 1), (1, 0), (1, 1))
                            ):
                                dst = qm[
                                    vy0 + 1 - dy : vy0 + 1 - dy + nr,
                                    1 - dx : 1 - dx + W,
                                    :,
                                    ci * D : (ci + 1) * D,
                                ]
                                nc.sync.dma_start(dst, vbt[:cw])
                    if band_mode:
                        # last band y=H uses V_H = 0
                        emit_band(H, vbt_prev, sh_prev, zt[:, :C], zt[:, :C])

            if debug:
                nc.sync.dma_start(dbg_vq[:], vq[:])

            # keepalive reads for partial (bisect) builds so dead-allocation
            # removal doesn't drop DRAM scratch still referenced by DMAs
            full = all(p in parts for p in ("value", "weights", "gather", "tail"))
            if not full:
                ka = cpool.tile([1, 4], f32, tag="ka", name="ka")
                kb = cpool.tile([1, 4], bf16, tag="kb", name="kb")
                nc.sync.dma_start(kb[0:1, 0:2], vq[1:2, 0, 0:2])
                nc.sync.dma_start(kb[0:1, 2:4], vq[0:1, 0, 0:2])
                nc.vector.tensor_copy(ka[:], kb[:])
                nc.sync.dma_start(outT[0:1, 0:4], ka[:])

            # ============ Phase 2: query supertiles ============
            # global Pool-DMA instruction counter: the tile framework assigns
            # DMASW sem lanes round-robin mod 8 over these instructions, and a
            # lane must stay on one SWDGE queue -> queue = counter % 4 keeps
            # lane l on queue l%4 consistently
            gq_counter = [0]
            st_off = 0
            for sti, qst in enumerate(supertiles if "query" in parts else []):
                QCh = qst // 128
                q_sl = slice(st_off, st_off + qst)

                # betaT/idx16/zf double-buffered so supertile i+1's weight
                # math can land while supertile i's gathers still read them
                zfT = stpool.tile([128, 2, qst], f32, tag="zfT", name="zfT")
                zfb = stpool.tile([128, 2, qst], bf16, tag="zfb", name="zfb",
                                  bufs=2)
                betaT = stpool.tile([128, QCh, 4, 128], bf16, tag="betaT",
                                    name="betaT", bufs=2)
                idx16 = stpool.tile([128, qst], i16, tag="idx16", name="idx16",
                                    bufs=2)
                acc = stpool.tile([128, QCh, M, D], f32, tag="acc", name="acc")
                accT = stpool.tile([128, 2, qst], bf16, tag="accT", name="accT")

                # ---- zf (feat+pos pre-added on host) ----
                nc.sync.dma_start(
                    zfT[:], zfTq[:, q_sl].rearrange("(co ci) t -> ci co t", ci=128)
                )
                nc.vector.tensor_copy(zfb[:], zfT[:])

                # ---- weight math: psum-coupled per-512 loop, then
                # full-supertile DVE ops with aggressive buffer reuse ----
                with (
                    tc.tile_pool(name="wm", bufs=1) as mp,
                    tc.tile_pool(name="psQ", bufs=2, space="PSUM") as psQ,
                    tc.tile_pool(name="psW", bufs=2, space="PSUM") as psW,
                ):
                  if "weights" in parts:
                    def ft(tag, dt=f32):
                        return mp.tile([128, qst], dt, tag=tag, name=tag)

                    bx, by, At = ft("bx"), ft("by"), ft("At")
                    r1, r2 = ft("r1"), ft("r2")
                    t1, t2, t3, t4 = ft("t1"), ft("t2"), ft("t3"), ft("t4")
                    V = nc.vector

                    for qq in range(0, qst, 512):
                        qw = min(512, qst - qq)
                        sl = slice(qq, qq + qw)
                        for dst_t, j0, bias_t in ((bx, 0, boffx), (by, 128, boffy)):
                            ps = psQ.tile([128, 512], f32, tag="psq", name="psq")
                            for co in range(2):
                                nc.tensor.matmul(
                                    ps[:, :qw], woff[:, co, j0 : j0 + 128],
                                    zfb[:, co, sl], start=(co == 0), stop=(co == 1),
                                )
                            nc.scalar.activation(
                                dst_t[:, sl], ps[:, :qw], AF.Identity, bias=bias_t[:]
                            )
                        ps = psQ.tile([128, 512], f32, tag="psq", name="psq")
                        for co in range(2):
                            nc.tensor.matmul(
                                ps[:, :qw], wattn[:, co, :], zfb[:, co, sl],
                                start=(co == 0), stop=(co == 1),
                            )
                        nc.scalar.activation(At[:, sl], ps[:, :qw], AF.Exp, bias=battn[:])
                        gs = psW.tile([8, 512], f32, tag="gs", name="gs")
                        nc.tensor.matmul(gs[:, :qw], sones[:], At[:, sl])
                        rgs = mp.tile([8, 512], f32, tag="rgs", name="rgs")
                        nc.vector.reciprocal(rgs[:, :qw], gs[:, :qw])
                        rb = psW.tile([128, 512], f32, tag="rb", name="rb")
                        nc.tensor.matmul(rb[:, :qw], sblk[:], rgs[:, :qw])
                        V.tensor_tensor(At[:, sl], At[:, sl], rb[:, :qw], AL.mult)

                    # refs (full supertile)
                    nc.sync.dma_start(r1[:], refxb_d[:, q_sl])
                    nc.sync.dma_start(r2[:], refyb_d[:, q_sl])
                    # px/py
                    V.scalar_tensor_tensor(bx[:], r1[:], W_row, bx[:], AL.mult, AL.add)
                    V.scalar_tensor_tensor(by[:], r2[:], H_row, by[:], AL.mult, AL.add)
                    BIG = float(3 << 22)
                    # x0f -> t1 (round(px-0.5) via magic adds), wx -> r1
                    V.tensor_scalar(t1[:], bx[:], -0.5, None, AL.add)
                    V.tensor_scalar(t1[:], t1[:], BIG, None, AL.add)
                    V.tensor_scalar(t1[:], t1[:], -BIG, None, AL.add)
                    V.tensor_tensor(r1[:], bx[:], t1[:], AL.subtract)
                    # y0f -> t2, wy -> r2
                    V.tensor_scalar(t2[:], by[:], -0.5, None, AL.add)
                    V.tensor_scalar(t2[:], t2[:], BIG, None, AL.add)
                    V.tensor_scalar(t2[:], t2[:], -BIG, None, AL.add)
                    V.tensor_tensor(r2[:], by[:], t2[:], AL.subtract)
                    # mx0 -> bx, mx1 -> by
                    V.tensor_scalar(bx[:], t1[:], 0.0, None, AL.is_ge)
                    V.tensor_scalar(t3[:], t1[:], Wm1_row, None, AL.is_le)
                    V.tensor_tensor(bx[:], bx[:], t3[:], AL.mult)
                    V.tensor_scalar(by[:], t1[:], -1.0, None, AL.is_ge)
                    V.tensor_scalar(t3[:], t1[:], Wm2_row, None, AL.is_le)
                    V.tensor_tensor(by[:], by[:], t3[:], AL.mult)
                    # u0 -> bx, u1 -> by
                    V.tensor_scalar(t3[:], r1[:], -1.0, 1.0, AL.mult, AL.add)
                    V.tensor_tensor(bx[:], t3[:], bx[:], AL.mult)
                    V.tensor_tensor(by[:], r1[:], by[:], AL.mult)
                    # my0 -> r1, my1 -> t4
                    V.tensor_scalar(r1[:], t2[:], 0.0, None, AL.is_ge)
                    V.tensor_scalar(t3[:], t2[:], Hm1_row, None, AL.is_le)
                    V.tensor_tensor(r1[:], r1[:], t3[:], AL.mult)
                    V.tensor_scalar(t4[:], t2[:], -1.0, None, AL.is_ge)
                    V.tensor_scalar(t3[:], t2[:], Hm2_row, None, AL.is_le)
                    V.tensor_tensor(t4[:], t4[:], t3[:], AL.mult)
                    # v0 -> r1, v1 -> t4
                    V.tensor_scalar(t3[:], r2[:], -1.0, 1.0, AL.mult, AL.add)
                    V.tensor_tensor(r1[:], t3[:], r1[:], AL.mult)
                    V.tensor_tensor(t4[:], r2[:], t4[:], AL.mult)
                    # betas (bf16) and transposes into betaT
                    bbs = []
                    for ci, (uu, vv) in enumerate(
                        ((bx, r1), (by, r1), (bx, t4), (by, t4))
                    ):
                        bb = mp.tile([128, qst], bf16, tag=f"bb{ci}", name=f"bb{ci}")
                        V.tensor_tensor(t3[:], uu[:], vv[:], AL.mult)
                        V.tensor_tensor(bb[:], t3[:], At[:], AL.mult)
                        bbs.append(bb)
                    if "w_beta" in parts:
                        for ci in range(4):
                            for qc in range(QCh):
                                pst = psW.tile([128, 128], bf16, tag="pst", name="pst")
                                nc.tensor.transpose(
                                    pst[:], bbs[ci][:, qc * 128 : (qc + 1) * 128],
                                    ident_bf[:],
                                )
                                nc.scalar.copy(betaT[:, qc, ci, :], pst[:])
                    # x0p -> t1, y0p -> t2, idx
                    V.tensor_scalar(t1[:], t1[:], 1.0, 0.0, AL.add, AL.max)
                    V.tensor_scalar(t1[:], t1[:], W_row, None, AL.min)
                    V.tensor_scalar(t2[:], t2[:], 1.0, 0.0, AL.add, AL.max)
                    V.tensor_scalar(t2[:], t2[:], H_row, None, AL.min)
                    V.scalar_tensor_tensor(t3[:], t2[:], Wp1_row, t1[:], AL.mult, AL.add)
                    V.tensor_copy(idx16[:], t3[:])

                # ---- gather + combine per (level, head) ----
                nc.vector.memset(acc[:], 0.0)
                JJ = 4 * qst
                FF = JJ // 16          # idx cols per (lv, m)
                RL = 8 * FF            # dlin rows per level (m-major, then k, f)
                if "gather" in parts:
                    idxd = dpool.tile([128, qst], i16, tag="idxd", name="idxd")
                    nc.sync.dma_start(idxd[:], idx16[:])
                    # 16-wrapped index image, all (lv, m) pairs. The SWDGE
                    # ucode pair for queue q only reads idx partitions
                    # 32q..32q+31 (replicas r=2q,2q+1); CoreSim always checks
                    # r=0,1. Write only those replicas instead of all 8.
                    # (DMA AP balancer caps at 3 dims -> one DMA per slice)
                    dlin_all = dpool.tile([L, RL, 128], i16, tag="dlin", name="dlin")
                    for lv in range(L):
                        for m in range(M):
                            s0 = m * 16 + lv * 4
                            srcb = idxd[s0 : s0 + 4].rearrange(
                                "k (f ql) -> (k f) ql", ql=16
                            )
                            dst3 = dlin_all[lv, m * FF : (m + 1) * FF].rearrange(
                                "f (r ql) -> f r ql", r=8
                            )
                            # queues 0-3 rotate per call: all 8 replicas
                            nc.sync.dma_start(
                                dst3, srcb[:, None, :].to_broadcast((FF, 8, 16))
                            )
                with tc.tile_pool(name="gp", bufs=2) as gp:
                    for lv in range(L if "gather" in parts else 0):
                        idxw = gp.tile([128, RL], i16, tag="idxw", name="idxw",
                                       bufs=2)
                        nc.sync.dma_start_transpose(idxw[:], dlin_all[lv])
                        for m in range(M):
                            s0 = m * 16 + lv * 4
                            ib = m * FF
                            g = gp.tile([128, 4 * QCh, 128], bf16, tag="g", name="g")
                            # SWDGE descriptor ring holds 1024 descs: split
                            # into <=1024-index sub-calls (128-aligned),
                            # rotating across the 4 SWDGE queues.
                            for c0 in range(0, JJ, 1024):
                                n_i = min(1024, JJ - c0)
                                nc.gpsimd.dma_gather(
                                    out_ap=g[:, c0 // 128 : (c0 + n_i) // 128, :],
                                    in_ap=vq[q_base[lv] : q_base[lv] + q_rows[lv], m, :],
                                    idxs_ap=idxw[:, ib + c0 // 16 : ib + (c0 + n_i) // 16],
                                    num_idxs=n_i,
                                    num_idxs_reg=n_i,
                                    elem_size=128,
                                    elem_step=M * 128,
                                    queue_num=gq_counter[0] % 4,
                                )
                                gq_counter[0] += 1
                            if debug and sti == 0 and lv == 0 and m == 0:
                                nc.sync.dma_start(dbg_g[:, : 4 * QCh, :], g[:])
                            gv = g[:].rearrange(
                                "p (k qc) (c d) -> p k qc c d", k=4, d=D
                            )
                            bt = betaT[:, :, :, s0 : s0 + 4]
                            btv = bt.rearrange("p qc c k -> p k qc c")[
                                :, :, :, :, None
                            ].to_broadcast((128, 4, QCh, 4, D))
                            tmp = gp.tile([128, 4, QCh, 4, D], bf16, tag="tmp", name="tmp", bufs=1)
                            nc.vector.tensor_tensor(tmp[:], gv, btv, AL.mult)
                            s1 = gp.tile([128, 4, QCh, 2, D], bf16, tag="s1", name="s1", bufs=1)
                            nc.vector.tensor_tensor(
                                s1[:], tmp[:, :, :, 0:4:2, :], tmp[:, :, :, 1:4:2, :],
                                AL.add,
                            )
                            s2 = gp.tile([128, 4, QCh, D], bf16, tag="s2", name="s2", bufs=1)
                            nc.vector.tensor_tensor(
                                s2[:], s1[:, :, :, 0, :], s1[:, :, :, 1, :], AL.add
                            )
                            s3 = gp.tile([128, 2, QCh, D], bf16, tag="s3", name="s3", bufs=1)
                            nc.vector.tensor_tensor(
                                s3[:], s2[:, 0:4:2], s2[:, 1:4:2], AL.add
                            )
                            s4 = gp.tile([128, QCh, D], f32, tag="s4", name="s4", bufs=1)
                            nc.vector.tensor_tensor(s4[:], s3[:, 0], s3[:, 1], AL.add)
                            nc.vector.tensor_tensor(
                                acc[:, :, m, :], acc[:, :, m, :], s4[:], AL.add
                            )

                if debug and sti == 0:
                    nc.sync.dma_start(dbg_idx[:, :qst], idx16[:])
                    nc.sync.dma_start(dbg_beta[:, :QCh], betaT[:])
                    nc.sync.dma_start(dbg_acc[:, :QCh], acc[:])

                # ---- transpose acc to channel-major bf16 ----
                with tc.tile_pool(name="psX", bufs=2, space="PSUM") as psX:
                    accv = acc[:].rearrange("p qc m d -> p qc (m d)")
                    for qc in range(QCh if "tail" in parts else 0):
                        for jb in range(2):
                            pst2 = psX.tile([128, 128], f32, tag="pst2", name="pst2")
                            nc.tensor.transpose(
                                pst2[:], accv[:, qc, jb * 128 : (jb + 1) * 128],
                                ident_f32[:],
                            )
                            nc.scalar.copy(
                                accT[:, jb, qc * 128 : (qc + 1) * 128], pst2[:]
                            )

                # ---- out proj + residual + LN1 + FFN + LN2 ----
                with (
                    tc.tile_pool(name="fp", bufs=2) as fp,
                    tc.tile_pool(name="lnp", bufs=1) as lp,
                    tc.tile_pool(name="psF", bufs=3, space="PSUM") as psF,
                    tc.tile_pool(name="psL", bufs=1, space="PSUM") as psL,
                ):
                    def layernorm(x_t, g_col, be_col, dst_f32, dst_bf, qw):
                        """x_t: [128, 2, qw] fp32 -> dst tiles [128, 2, qw]."""
                        mu = psL.tile([1, 512], f32, tag="mu", name="mu")
                        for co in range(2):
                            nc.tensor.matmul(
                                mu[:, :qw], ones_col[:], x_t[:, co, :qw],
                                start=(co == 0), stop=(co == 1),
                            )
                        mus = lp.tile([1, 512], f32, tag="mus", name="mus")
                        nc.scalar.activation(
                            mus[:, :qw], mu[:, :qw], AF.Identity, scale=1.0 / C
                        )
                        mub = psL.tile([128, 512], f32, tag="mub", name="mub")
                        nc.tensor.matmul(mub[:, :qw], ones_row[:], mus[:, :qw])
                        xc = lp.tile([128, 2, 512], f32, tag="xc", name="xc")
                        sq = lp.tile([128, 2, 512], f32, tag="sq", name="sq")
                        for co in range(2):
                            nc.vector.tensor_tensor(
                                xc[:, co, :qw], x_t[:, co, :qw], mub[:, :qw],
                                AL.subtract,
                            )
                            nc.scalar.activation(
                                sq[:, co, :qw], xc[:, co, :qw], AF.Square
                            )
                        var = psL.tile([1, 512], f32, tag="var", name="var")
                        for co in range(2):
                            nc.tensor.matmul(
                                var[:, :qw], ones_col[:], sq[:, co, :qw],
                                start=(co == 0), stop=(co == 1),
                            )
                        sd = lp.tile([1, 512], f32, tag="sd", name="sd")
                        nc.scalar.activation(
                            sd[:, :qw], var[:, :qw], AF.Sqrt, bias=eps1[:], scale=1.0 / C
                        )
                        rsd = lp.tile([1, 512], f32, tag="rsd", name="rsd")
                        nc.vector.reciprocal(rsd[:, :qw], sd[:, :qw])
                        isb = psL.tile([128, 512], f32, tag="isb", name="isb")
                        nc.tensor.matmul(isb[:, :qw], ones_row[:], rsd[:, :qw])
                        for co in range(2):
                            nc.vector.tensor_tensor(
                                xc[:, co, :qw], xc[:, co, :qw], isb[:, :qw], AL.mult
                            )
                            nc.vector.tensor_scalar(
                                dst_f32[:, co, :qw], xc[:, co, :qw],
                                g_col[:, co : co + 1], be_col[:, co : co + 1],
                                AL.mult, AL.add,
                            )
                            if dst_bf is not None:
                                nc.vector.tensor_copy(
                                    dst_bf[:, co, :qw], dst_f32[:, co, :qw]
                                )

                    for qq in range(0, qst if "tail" in parts else 0, 512):
                        qw = min(512, qst - qq)
                        sl = slice(qq, qq + qw)
                        # x = zf + acc @ W_out + b_out
                        xT_t = fp.tile([128, 2, 512], f32, tag="xT_t", name="xT_t")
                        for jb in range(2):
                            ps = psF.tile([128, 512], f32, tag="psf", name="psf")
                            for co in range(2):
                                nc.tensor.matmul(
                                    ps[:, :qw],
                                    wout[:, co, jb * 128 : (jb + 1) * 128],
                                    accT[:, co, sl],
                                    start=(co == 0), stop=(co == 1),
                                )
                            nc.vector.scalar_tensor_tensor(
                                xT_t[:, jb, :qw], ps[:, :qw],
                                bout_t[:, jb : jb + 1], zfT[:, jb, sl],
                                AL.add, AL.add,
                            )
                        x1 = fp.tile([128, 2, 512], f32, tag="x1", name="x1")
                        x1b = fp.tile([128, 2, 512], bf16, tag="x1b", name="x1b")
                        layernorm(xT_t, g1_t, be1_t, x1, x1b, qw)

                        hb = fp.tile([128, 16, 512], bf16, tag="hb", name="hb")
                        for jb in range(16):
                            ps = psF.tile([128, 512], f32, tag="psf", name="psf")
                            for co in range(2):
                                nc.tensor.matmul(
                                    ps[:, :qw],
                                    w1[:, co, jb * 128 : (jb + 1) * 128],
                                    x1b[:, co, :qw],
                                    start=(co == 0), stop=(co == 1),
                                )
                            nc.scalar.activation(
                                hb[:, jb, :qw], ps[:, :qw], AF.Relu,
                                bias=b1_t[:, jb : jb + 1],
                            )
                        x2 = fp.tile([128, 2, 512], f32, tag="x2", name="x2")
                        for jb in range(2):
                            ps = psF.tile([128, 512], f32, tag="psf", name="psf")
                            for kb in range(16):
                                nc.tensor.matmul(
                                    ps[:, :qw],
                                    w2[:, kb, jb * 128 : (jb + 1) * 128],
                                    hb[:, kb, :qw],
                                    start=(kb == 0), stop=(kb == 15),
                                )
                            nc.vector.scalar_tensor_tensor(
                                x2[:, jb, :qw], ps[:, :qw], b2_t[:, jb : jb + 1],
                                x1[:, jb, :qw], AL.add, AL.add,
                            )
                        out5 = fp.tile([128, 2, 512], f32, tag="out5", name="out5")
                        layernorm(x2, g2_t, be2_t, out5, None, qw)
                        nc.sync.dma_start(
                            outT[:, st_off + qq : st_off + qq + qw].rearrange(
                                "(co ci) t -> ci co t", ci=128
                            ),
                            out5[:, :, :qw],
                        )

                st_off += qst

    nc.finalize()
    return nc


# ======================= host side =======================

def _prep_core_inputs(inputs, b, s, sizes=None, qp=None):
    """Build the per-core input map (numpy) for batch b, query shard s."""
    if sizes is None:
        sizes = SIZES
    if qp is None:
        qp = QP
    hwl, ntok, lvl_base, *_ = _geom(sizes)
    nl = len(sizes)

    feats = [np.asarray(inputs[f"feat{i}"]) for i in range(nl)]
    poss = [np.asarray(inputs[f"pos{i}"]) for i in range(nl)]
    refs = [np.asarray(inputs[f"ref{i}"]) for i in range(nl)]

    x_all = np.concatenate([f[b].reshape(-1, C) for f in feats], 0)   # [ntok, C]
    p_all = np.concatenate([p[b].reshape(-1, C) for p in poss], 0)
    xT = np.ascontiguousarray((x_all + p_all).T.astype(F32))
    xTb = xT.astype(BF16)

    own = []
    for i in range(nl):
        n4 = hwl[i] // QSHARDS
        own.append(np.arange(lvl_base[i] + s * n4, lvl_base[i] + (s + 1) * n4))
    own = np.concatenate(own)
    nq = own.shape[0]

    zfTq = np.zeros((C, qp), F32)
    zfTq[:, :nq] = xT[:, own]

    ref_all = np.concatenate([r[b].reshape(-1, 2) for r in refs], 0)
    refq = np.zeros((qp, 2), F32)
    refq[:nq] = ref_all[own]
    refxb = np.ascontiguousarray(np.broadcast_to(refq[:, 0], (128, qp))).astype(F32)
    refyb = np.ascontiguousarray(np.broadcast_to(refq[:, 1], (128, qp))).astype(F32)

    consts = np.zeros((128, 8), F32)
    for sr in range(128):
        lvl = (sr // KPT) % len(sizes)
        H, W = sizes[lvl]
        consts[sr] = [W, H, W + 1, W - 1, H - 1, W - 2, H - 2, 0]

    def t_in(w):  # [C, N] -> [128, 2, N] (ci, co, n) in bf16
        w = np.asarray(w)
        return np.ascontiguousarray(
            w.reshape(2, 128, -1).transpose(1, 0, 2)
        ).astype(BF16)

    W_off = np.asarray(inputs["W_off"]).reshape(C, M, L, KPT, 2)
    W_off_p = W_off.transpose(0, 4, 1, 2, 3).reshape(C, C)   # j' = c*128 + (m,l,k)
    b_off = np.asarray(inputs["b_off"]).reshape(M, L, KPT, 2)
    b_off_p = b_off.transpose(3, 0, 1, 2).reshape(C)

    w2 = np.asarray(inputs["W2"])
    w2_t = np.ascontiguousarray(w2.reshape(16, 128, C).transpose(1, 0, 2)).astype(BF16)

    col2 = lambda v: np.ascontiguousarray(np.asarray(v).reshape(2, 128).T).astype(F32)
    sones = np.zeros((128, 8), F32)
    for sr in range(128):
        sones[sr, sr // 16] = 1.0
    sblk = np.ascontiguousarray(sones.T).astype(F32)

    return {
        "xTb": xTb, "zfTq": zfTq,
        "shup": np.eye(128, k=1).astype(BF16),
        "refxb": refxb, "refyb": refyb, "consts": consts,
        "wval": t_in(inputs["W_val"]), "woff": t_in(W_off_p),
        "wattn": t_in(inputs["W_attn"]), "wout": t_in(inputs["W_out"]),
        "w1": t_in(inputs["W1"]), "w2": w2_t,
        "bval_bc": np.ascontiguousarray(
            np.broadcast_to(np.asarray(inputs["b_val"]), (128, C))).astype(F32),
        "boffx": np.ascontiguousarray((b_off_p[:128] - 0.5).reshape(128, 1)).astype(F32),
        "boffy": np.ascontiguousarray((b_off_p[128:] - 0.5).reshape(128, 1)).astype(F32),
        "battn": np.ascontiguousarray(
            np.asarray(inputs["b_attn"]).reshape(128, 1)).astype(F32),
        "sones": sones, "sblk": sblk,
        "bout": col2(inputs["b_out"]),
        "b1": np.ascontiguousarray(
            np.asarray(inputs["b1"]).reshape(16, 128).T).astype(F32),
        "b2": col2(inputs["b2"]),
        "g1": col2(inputs["g1"]), "be1": col2(inputs["be1"]),
        "g2": col2(inputs["g2"]), "be2": col2(inputs["be2"]),
    }, own, nq


_NC_CACHE = {}


def get_program():
    if "main" not in _NC_CACHE:
        _NC_CACHE["main"] = build_program()
    return _NC_CACHE["main"]


def kernel(**inputs):
    from concourse.bass_utils import run_bass_kernel_spmd

    nc = get_program()
    in_maps = []
    metas = []
    for c in range(NCORES):
        b, s = c // QSHARDS, c % QSHARDS
        im, own, nq = _prep_core_inputs(inputs, b, s)
        in_maps.append(im)
        metas.append((b, own, nq))

    res = run_bass_kernel_spmd(nc, in_maps, core_ids=list(range(NCORES)))

    out = np.zeros((B, NTOK, C), F32)
    for c in range(NCORES):
        b, own, nq = metas[c]
        outT = res.results[c]["outT"]          # [C, QP]
        out[b, own, :] = outT[:, :nq].T
    return out

